# revision 1
# baseline (speedup 1.0000x reference)
"""MoE routing layer on 8 Trainium2 NeuronCores (data-parallel over batch).

Per core (4 samples):
  routing MLP -> cosine sim vs embeddings -> softmax weights wf[4,10]
  w_eff[b] = sum_n wf[b,n] * conv_w[n]  (conv is linear in weights ->
  10x fewer conv FLOPs than materializing all expert convs)
  out[b] = conv2d(x[b], w_eff[b]) + b_eff[b]

Conv is 9 shifted matmuls over the flat 58-wide grid (tap = constant
free-dim offset); two samples run concurrently on the PE array via
row tiling (partitions 0-63 / 64-127), fp32r for full-rate streaming.
"""
import sys

sys.path.insert(0, "/opt/trn_rl_repo")

import numpy as np

import concourse.bass as bass
import concourse.mybir as mybir
from concourse.masks import make_identity
from concourse.tile import TileContext

F32 = mybir.dt.float32
F32R = mybir.dt.float32r
AF = mybir.ActivationFunctionType
ALU = mybir.AluOpType
AX = mybir.AxisListType

NCORES = 8
BLOC = 4           # samples per core
CIN = 64
COUT = 64
H = W = 58
HW = H * W         # 3364
OH = OW = 56
NB = 10            # experts
EDIM = 64
RSIZE = 512
HID = 128
NTAP = 9
CHUNK_ROWS = 8
NCHUNK = 7         # 7*8 = 56 output rows
NFREE = CHUNK_ROWS * W  # 464 <= 512 (one PSUM bank)
TAP_OFF = [dy * W + dx for dy in range(3) for dx in range(3)]
PAIRED = True      # 2-sample row-tiled PE packing
CONV_DT = F32R


def fix_sync_waits(nc, cap=2):
    """This walrus build allows at most `cap` sem waits per instruction.
    Splice same-engine NoOps carrying the excess waits right before any
    over-subscribed instruction (waits happen earlier => same semantics)."""
    uid = [0]
    for f in nc.m.functions:
        for blk in f.blocks:
            insts = blk.instructions  # live list
            i = 0
            while i < len(insts):
                inst = insts[i]
                si = inst.sync_info
                waits = list(si.on_wait) if si and si.on_wait else []
                icap = 1
                if len(waits) <= icap:
                    i += 1
                    continue
                keep, excess = waits[-icap:], waits[:-icap]
                for k in range(0, len(excess), icap):
                    nop = mybir.InstNoOp(
                        name=f"{inst.name}-wsplit{uid[0]}", ins=[], outs=[]
                    )
                    uid[0] += 1
                    nop.engine = inst.engine
                    nop.sync_info = mybir.SyncInfo(
                        on_wait=excess[k : k + icap], on_update=[]
                    )
                    nc.register_instruction(nop, overwrite=True)
                    insts.insert(i, nop)
                    i += 1
                inst.sync_info = mybir.SyncInfo(
                    on_wait=keep,
                    on_update=list(si.on_update) if si and si.on_update else [],
                )
                i += 1


def build():
    nc = bass.Bass(num_swdge_queues=4)
    x = nc.dram_tensor("x", [BLOC, CIN, H, W], F32, kind="ExternalInput")
    rv = nc.dram_tensor("rv", [BLOC, RSIZE], F32, kind="ExternalInput")
    w1 = nc.dram_tensor("w1", [RSIZE, HID], F32, kind="ExternalInput")
    bias1 = nc.dram_tensor("bias1", [HID, 1], F32, kind="ExternalInput")
    w2 = nc.dram_tensor("w2", [HID, EDIM], F32, kind="ExternalInput")
    bias2 = nc.dram_tensor("bias2", [EDIM, 1], F32, kind="ExternalInput")
    emb = nc.dram_tensor("emb", [NB, EDIM], F32, kind="ExternalInput")
    cwp = nc.dram_tensor("cwp", [CIN, NB, NTAP, COUT], F32, kind="ExternalInput")
    cb = nc.dram_tensor("cb", [NB, COUT], F32, kind="ExternalInput")
    sel = nc.dram_tensor("sel", [2, BLOC, 128], F32, kind="ExternalInput")
    identin = nc.dram_tensor("identin", [128, 128], F32, kind="ExternalInput")
    out = nc.dram_tensor("out", [BLOC, COUT, OH, OW], F32, kind="ExternalOutput")

    with TileContext(nc) as tc:
        with (
            tc.tile_pool(name="consts", bufs=1) as consts,
            tc.tile_pool(name="work", bufs=2) as work,
            tc.tile_pool(name="stage", bufs=4) as stage,
            tc.tile_pool(name="ps", bufs=2, space="PSUM") as pspool,
            tc.tile_pool(name="psconv", bufs=2, space="PSUM") as psconv,
        ):
            # ---------- inputs / constants into SBUF ----------
            ident = consts.tile([128, 128], F32, tag="ident")
            nc.sync.dma_start(out=ident[:], in_=identin[:])
            ones64 = consts.tile([EDIM, 1], F32, tag="ones64")
            nc.vector.memset(ones64[:], 1.0)

            rvsb = consts.tile([BLOC, RSIZE], F32, tag="rvsb")
            nc.sync.dma_start(out=rvsb[:], in_=rv[:])
            w1sb = consts.tile([128, 4, HID], F32, tag="w1sb")
            nc.sync.dma_start(
                out=w1sb[:], in_=w1[:].rearrange("(c k) m -> k c m", k=128)
            )
            w2sb = consts.tile([HID, EDIM], F32, tag="w2sb")
            nc.sync.dma_start(out=w2sb[:], in_=w2[:])
            b1sb = consts.tile([HID, 1], F32, tag="b1sb")
            nc.sync.dma_start(out=b1sb[:], in_=bias1[:])
            b2sb = consts.tile([EDIM, 1], F32, tag="b2sb")
            nc.sync.dma_start(out=b2sb[:], in_=bias2[:])
            embsb = consts.tile([NB, EDIM], F32, tag="embsb")
            nc.sync.dma_start(out=embsb[:], in_=emb[:])
            cbsb = consts.tile([NB, COUT], F32, tag="cbsb")
            nc.sync.dma_start(out=cbsb[:], in_=cb[:])
            selsb = consts.tile([BLOC, 2, 128], F32, tag="selsb")
            nc.sync.dma_start(out=selsb[:], in_=sel[:].rearrange("j b p -> b j p"))

            cwp2 = consts.tile([128, NB, NTAP, COUT], F32, tag="cwp2")
            nc.sync.dma_start(out=cwp2[0:64], in_=cwp[:])
            nc.sync.dma_start(out=cwp2[64:128], in_=cwp[:])

            xt = []
            for j in range(2):
                t = consts.tile([128, HW + 4], CONV_DT, tag=f"xt{j}")
                nc.vector.memset(t[:, HW : HW + 4].bitcast(F32), 0.0)
                nc.gpsimd.dma_start(
                    out=t[0:64, 0:HW], in_=x[2 * j].rearrange("c h w -> c (h w)")
                )
                nc.gpsimd.dma_start(
                    out=t[64:128, 0:HW],
                    in_=x[2 * j + 1].rearrange("c h w -> c (h w)"),
                )
                xt.append(t)

            # ---------- routing MLP ----------
            # rv [4, 512] -> rvT [128, 4(chunk), 4(sample)] via PE transposes
            rvT = work.tile([128, 4, BLOC], F32, tag="rvT")
            for c in range(4):
                pst = pspool.tile([128, BLOC], F32, tag="small")
                nc.tensor.transpose(
                    pst[:], rvsb[:, c * 128 : (c + 1) * 128], ident[0:BLOC, 0:BLOC]
                )
                nc.scalar.copy(out=rvT[:, c, :], in_=pst[:])
            h1 = pspool.tile([HID, BLOC], F32, tag="small")
            for c in range(4):
                nc.tensor.matmul(
                    h1[:], w1sb[:, c, :], rvT[:, c, :], start=(c == 0), stop=(c == 3)
                )
            h1r = work.tile([HID, BLOC], F32, tag="h1r")
            nc.scalar.activation(
                out=h1r[:], in_=h1[:], func=AF.Relu, bias=b1sb[:], scale=1.0
            )
            rps = pspool.tile([EDIM, BLOC], F32, tag="small")
            nc.tensor.matmul(rps[:], w2sb[:], h1r[:], start=True, stop=True)
            rsb = work.tile([EDIM, BLOC], F32, tag="rsb")
            nc.scalar.activation(
                out=rsb[:], in_=rps[:], func=AF.Identity, bias=b2sb[:], scale=1.0
            )

            # ---------- cosine similarity ----------
            rsq = work.tile([EDIM, BLOC], F32, tag="rsq")
            nc.vector.tensor_mul(rsq[:], rsb[:], rsb[:])
            nsq = pspool.tile([BLOC, 1], F32, tag="small")
            nc.tensor.matmul(nsq[:], rsq[:], ones64[:], start=True, stop=True)
            rln = work.tile([BLOC, 1], F32, tag="rln")
            nc.scalar.activation(out=rln[:], in_=nsq[:], func=AF.Ln)
            rinv = work.tile([BLOC, 1], F32, tag="rinv")
            nc.scalar.activation(out=rinv[:], in_=rln[:], func=AF.Exp, scale=-0.5)

            esq = work.tile([NB, EDIM], F32, tag="esq")
            nc.vector.tensor_mul(esq[:], embsb[:], embsb[:])
            ensq = work.tile([NB, 1], F32, tag="ensq")
            nc.vector.tensor_reduce(ensq[:], esq[:], axis=AX.X, op=ALU.add)
            eln = work.tile([NB, 1], F32, tag="eln")
            nc.scalar.activation(out=eln[:], in_=ensq[:], func=AF.Ln)
            einv = work.tile([NB, 1], F32, tag="einv")
            nc.scalar.activation(out=einv[:], in_=eln[:], func=AF.Exp, scale=-0.5)
            embn = work.tile([NB, EDIM], F32, tag="embn")
            nc.vector.tensor_scalar_mul(out=embn[:], in0=embsb[:], scalar1=einv[:])
            embnT_ps = pspool.tile([EDIM, NB], F32, tag="small")
            nc.tensor.transpose(embnT_ps[:], embn[:], ident[0:NB, 0:NB])
            embnT = work.tile([EDIM, NB], F32, tag="embnT")
            nc.scalar.copy(out=embnT[:], in_=embnT_ps[:])

            simps = pspool.tile([BLOC, NB], F32, tag="small")
            nc.tensor.matmul(simps[:], rsb[:], embnT[:], start=True, stop=True)
            sim = work.tile([BLOC, NB], F32, tag="sim")
            nc.vector.tensor_scalar_mul(out=sim[:], in0=simps[:], scalar1=rinv[:])

            # ---------- softmax ----------
            mx = work.tile([BLOC, 1], F32, tag="mx")
            nc.vector.tensor_reduce(mx[:], sim[:], axis=AX.X, op=ALU.max)
            negmx = work.tile([BLOC, 1], F32, tag="negmx")
            nc.vector.tensor_scalar_mul(out=negmx[:], in0=mx[:], scalar1=-1.0)
            ex = work.tile([BLOC, NB], F32, tag="ex")
            nc.scalar.activation(
                out=ex[:], in_=sim[:], func=AF.Exp, bias=negmx[:], scale=1.0
            )
            s = work.tile([BLOC, 1], F32, tag="s")
            nc.vector.tensor_reduce(s[:], ex[:], axis=AX.X, op=ALU.add)
            sinv = work.tile([BLOC, 1], F32, tag="sinv")
            nc.vector.reciprocal(sinv[:], s[:])
            wf = work.tile([BLOC, NB], F32, tag="wf")
            nc.vector.tensor_scalar_mul(out=wf[:], in0=ex[:], scalar1=sinv[:])

            # ---------- effective conv bias ----------
            wfT_ps = pspool.tile([NB, BLOC], F32, tag="small")
            nc.tensor.transpose(wfT_ps[:], wf[:], ident[0:BLOC, 0:BLOC])
            wfT = work.tile([NB, BLOC], F32, tag="wfT")
            nc.scalar.copy(out=wfT[:], in_=wfT_ps[:])
            beff_ps = pspool.tile([COUT, BLOC], F32, tag="small")
            nc.tensor.matmul(beff_ps[:], cbsb[:], wfT[:], start=True, stop=True)
            beff = work.tile([COUT, BLOC], F32, tag="beff")
            nc.scalar.copy(out=beff[:], in_=beff_ps[:])

            # ---------- PE warmup: keep HAM busy until conv starts ----------
            warm_ps = pspool.tile([128, 512], F32, tag="warm")
            wl = ident[:].bitcast(mybir.dt.bfloat16)[:, 0:128]
            wr = w1sb[:].rearrange("p c m -> p (c m)").bitcast(mybir.dt.bfloat16)[:, 0:512]
            for _ in range(22):
                nc.tensor.matmul(warm_ps[:], wl, wr, start=True, stop=True)
            warm_sink = work.tile([1, 1], F32, tag="warm_sink")
            nc.scalar.copy(out=warm_sink[:], in_=warm_ps[0:1, 0:1])

            # ---------- both pairs: weights broadcast + w_eff first ----------
            weffs = []
            for j in range(2):
                wfbc_ps = pspool.tile([128, NB], F32, tag="small")
                nc.tensor.matmul(
                    wfbc_ps[:], selsb[:, j, :], wf[:], start=True, stop=True
                )
                wfbc = work.tile([128, NB], F32, tag=f"wfbc{j}")
                nc.scalar.copy(out=wfbc[:], in_=wfbc_ps[:])

                weff = work.tile([128, NTAP, COUT], CONV_DT, tag=f"weff{j}")
                for lo, hi in ((0, 5), (5, NTAP)):
                    nc.vector.tensor_scalar_mul(
                        out=weff[:, lo:hi], in0=cwp2[:, 0, lo:hi], scalar1=wfbc[:, 0:1]
                    )
                    for n in range(1, NB):
                        nc.vector.scalar_tensor_tensor(
                            out=weff[:, lo:hi],
                            in0=cwp2[:, n, lo:hi],
                            scalar=wfbc[:, n : n + 1],
                            in1=weff[:, lo:hi],
                            op0=ALU.mult,
                            op1=ALU.add,
                        )
                weffs.append(weff)

            # ---------- PE warmup: keep HAM busy until conv starts ----------
            warm_ps = pspool.tile([128, 512], F32, tag="warm")
            wl = ident[:].bitcast(mybir.dt.bfloat16)[:, 0:128]
            wr = w1sb[:].rearrange("p c m -> p (c m)").bitcast(mybir.dt.bfloat16)[:, 0:512]
            for _ in range(22):
                nc.tensor.matmul(warm_ps[:], wl, wr, start=True, stop=True)
            warm_sink = work.tile([1, 1], F32, tag="warm_sink")
            nc.scalar.copy(out=warm_sink[:], in_=warm_ps[0:1, 0:1])

            # ---------- conv ----------
            for j in range(2):
                weff = weffs[j]
                for ch in range(NCHUNK):
                    h0 = ch * CHUNK_ROWS
                    psA = psconv.tile([COUT, NFREE], F32, tag="psA")
                    psB = psconv.tile([COUT, NFREE], F32, tag="psB")
                    for t in range(NTAP):
                        off = h0 * W + TAP_OFF[t]
                        nc.tensor.matmul(
                            psA[:],
                            weff[0:64, t, :],
                            xt[j][0:64, off : off + NFREE],
                            start=(t == 0),
                            stop=(t == NTAP - 1),
                            tile_position=(0, 0) if PAIRED else None,
                        )
                        nc.tensor.matmul(
                            psB[:],
                            weff[64:128, t, :],
                            xt[j][64:128, off : off + NFREE],
                            start=(t == 0),
                            stop=(t == NTAP - 1),
                            tile_position=(64, 0) if PAIRED else None,
                        )
                    for half, ps in ((0, psA), (1, psB)):
                        b = 2 * j + half
                        st = stage.tile([COUT, CHUNK_ROWS, OW], F32, tag="st")
                        psv = ps[:].rearrange("p (r w) -> p r w", w=W)[:, :, 0:OW]
                        nc.scalar.activation(
                            out=st[:],
                            in_=psv,
                            func=AF.Identity,
                            bias=beff[:, b : b + 1],
                            scale=1.0,
                        )
                        nc.sync.dma_start(
                            out=out[b, :, h0 : h0 + CHUNK_ROWS, :], in_=st[:]
                        )

    fix_sync_waits(nc)
    return nc


_NC = None


def _get_nc():
    global _NC
    if _NC is None:
        _NC = build()
    return _NC


def make_in_maps(inputs):
    x = np.ascontiguousarray(np.asarray(inputs["x"], dtype=np.float32))
    rvec = np.ascontiguousarray(np.asarray(inputs["routing_vector"], dtype=np.float32))
    W1 = np.ascontiguousarray(np.asarray(inputs["W1"], dtype=np.float32))
    b1 = np.ascontiguousarray(np.asarray(inputs["b1"], dtype=np.float32)).reshape(HID, 1)
    W2 = np.ascontiguousarray(np.asarray(inputs["W2"], dtype=np.float32))
    b2 = np.ascontiguousarray(np.asarray(inputs["b2"], dtype=np.float32)).reshape(EDIM, 1)
    emb = np.ascontiguousarray(np.asarray(inputs["emb"], dtype=np.float32))
    conv_w = np.asarray(inputs["conv_w"], dtype=np.float32)
    conv_b = np.ascontiguousarray(np.asarray(inputs["conv_b"], dtype=np.float32))
    # conv_w[n, co, ci, ky, kx] -> cwp[ci, n, (ky kx), co]
    cwpa = np.ascontiguousarray(
        conv_w.transpose(2, 0, 3, 4, 1).reshape(CIN, NB, NTAP, COUT)
    )
    selm = np.zeros((2, BLOC, 128), np.float32)
    for j in range(2):
        selm[j, 2 * j, 0:64] = 1.0
        selm[j, 2 * j + 1, 64:128] = 1.0
    identm = np.eye(128, dtype=np.float32)
    in_maps = []
    for c in range(NCORES):
        in_maps.append(
            {
                "x": np.ascontiguousarray(x[BLOC * c : BLOC * (c + 1)]),
                "rv": np.ascontiguousarray(rvec[BLOC * c : BLOC * (c + 1)]),
                "w1": W1,
                "bias1": b1,
                "w2": W2,
                "bias2": b2,
                "emb": emb,
                "cwp": cwpa,
                "cb": conv_b,
                "sel": selm,
                "identin": identm,
            }
        )
    return in_maps


def kernel(**inputs):
    from concourse.bass_utils import run_bass_kernel_spmd

    nc = _get_nc()
    in_maps = make_in_maps(inputs)
    res = run_bass_kernel_spmd(nc, in_maps, core_ids=list(range(NCORES)))
    return np.concatenate([r["out"] for r in res.results], axis=0)



# revision 4
# speedup vs baseline: 1.3951x; 1.3951x over previous
"""MoE routing layer on 8 Trainium2 NeuronCores (data-parallel over batch).

Per core (4 samples):
  routing MLP -> cosine sim vs embeddings -> softmax weights wf[4,10]
  w_eff[b] = sum_n wf[b,n] * conv_w[n]  (conv linear in weights ->
  10x fewer conv FLOPs than materializing all expert convs)
  out[b] = conv2d(x[b], w_eff[b]) + b_eff[b]

Conv is 9 shifted fp16 matmuls over the flat 58-wide grid; FOUR 64x64
PE quadrants run concurrently (2 row groups x 2 col groups), covering
2 samples x 2 row-chunks per slot. A 3-phase schedule lets pair-0 conv
start while pair-1's mixed weights are still being computed on DVE.
All inputs stream over hardware DGE queues; small consts are packed
into one blob descriptor. Output DMA writes directly in final layout.
"""
import sys

sys.path.insert(0, "/opt/trn_rl_repo")

import numpy as np

import concourse.bass as bass
import concourse.mybir as mybir
from concourse.tile import TileContext

F32 = mybir.dt.float32
F16 = mybir.dt.float16
AF = mybir.ActivationFunctionType
ALU = mybir.AluOpType
AX = mybir.AxisListType

NCORES = 8
BLOC = 4           # samples per core
CIN = 64
COUT = 64
H = W = 58
HW = H * W         # 3364
OH = OW = 56
NB = 10            # experts
EDIM = 64
RSIZE = 512
HID = 128
NTAP = 9
CHUNK = 8          # output rows per chunk
NCH = 7            # 7*8 = 56 output rows
NFREE = CHUNK * W  # 464 <= 512 (one PSUM bank)
TAP_OFF = [dy * W + dx for dy in range(3) for dx in range(3)]
NWARM = 8

# blob128 column layout (f32, [128, NCOL128])
C_W1 = 0                 # 512 cols: w1 as [128, 4, 128]
C_W2 = C_W1 + 512        # 64 cols
C_RVT = C_W2 + 64        # 16 cols: rvT as [128, 4, 4]
C_B1 = C_RVT + 16        # 1 col
C_B2 = C_B1 + 1          # 1 col (rows 0:64)
C_EMBT = C_B2 + 1        # 10 cols (rows 0:64)
C_ID4 = C_EMBT + 10      # 4 cols (rows 0:4 = eye(4))
NCOL128 = C_ID4 + 4

# blob10 column layout (f32, [10, 384])
# 0:128 = [cb | cb]; 128:256 = selA (rows 0:4); 256:384 = selB
SAMPLE_STRIDE = COUT * OH * OW  # 200704


def fix_sync_waits(nc, cap=2):
    """This walrus build allows at most `cap` sem waits per instruction.
    Splice same-engine NoOps carrying the excess waits right before any
    over-subscribed instruction (waits happen earlier => same semantics)."""
    uid = [0]
    for f in nc.m.functions:
        for blk in f.blocks:
            insts = blk.instructions  # live list
            i = 0
            while i < len(insts):
                inst = insts[i]
                si = inst.sync_info
                waits = list(si.on_wait) if si and si.on_wait else []
                icap = 1
                if len(waits) <= icap:
                    i += 1
                    continue
                keep, excess = waits[-icap:], waits[:-icap]
                for k in range(0, len(excess), icap):
                    nop = mybir.InstNoOp(
                        name=f"{inst.name}-wsplit{uid[0]}", ins=[], outs=[]
                    )
                    uid[0] += 1
                    nop.engine = inst.engine
                    nop.sync_info = mybir.SyncInfo(
                        on_wait=excess[k : k + icap], on_update=[]
                    )
                    nc.register_instruction(nop, overwrite=True)
                    insts.insert(i, nop)
                    i += 1
                inst.sync_info = mybir.SyncInfo(
                    on_wait=keep,
                    on_update=list(si.on_update) if si and si.on_update else [],
                )
                i += 1


def build():
    nc = bass.Bass(num_swdge_queues=4)
    x_d = nc.dram_tensor("x", [BLOC, CIN, HW], F16, kind="ExternalInput")
    cwp_d = nc.dram_tensor("cwp", [CIN, NB * NTAP * COUT], F16, kind="ExternalInput")
    blob128_d = nc.dram_tensor("blob128", [128, NCOL128], F32, kind="ExternalInput")
    blob10_d = nc.dram_tensor("blob10", [NB, 384], F32, kind="ExternalInput")
    out_d = nc.dram_tensor("out", [BLOC, COUT, OH, OW], F32, kind="ExternalOutput")

    with TileContext(nc) as tc:
        with (
            tc.tile_pool(name="consts", bufs=1) as consts,
            tc.tile_pool(name="work", bufs=2) as work,
            tc.tile_pool(name="stage", bufs=3) as stpool,
            tc.tile_pool(name="ps", bufs=2, space="PSUM") as pspool,
            tc.tile_pool(name="psconv", bufs=2, space="PSUM") as psconv,
            tc.tile_pool(name="pswarm", bufs=1, space="PSUM") as pswarm,
        ):
            # ---------- SBUF constants / inputs ----------
            ones64 = consts.tile([EDIM, 1], F32, tag="ones64")
            nc.vector.memset(ones64[:], 1.0)
            onesR = consts.tile([1, EDIM], F32, tag="onesR")
            nc.vector.memset(onesR[:], 1.0)

            blob128 = consts.tile([128, NCOL128], F32, tag="blob128")
            blob10 = consts.tile([NB, 384], F32, tag="blob10")
            cwp2 = consts.tile([128, NB, NTAP, COUT], F16, tag="cwp2")
            xt = []
            for j in range(2):
                t = consts.tile([128, HW + 4], F16, tag=f"xt{j}")
                nc.vector.memset(t[:, HW : HW + 4], 0.0)
                xt.append(t)

            # DMA dispatch order == queue order.
            # sync queue: blob128, cwp half 1, x0..x3 (then out descriptors)
            nc.sync.dma_start(out=blob128[:], in_=blob128_d[:])
            nc.sync.dma_start(
                out=cwp2[0:64].rearrange("p n t c -> p (n t c)"), in_=cwp_d[:]
            )
            for s in range(BLOC):
                half = s % 2
                nc.sync.dma_start(
                    out=xt[s // 2][64 * half : 64 * half + 64, 0:HW],
                    in_=x_d[s],
                )
            # scalar queue: blob10, cwp half 2
            nc.scalar.dma_start(out=blob10[:], in_=blob10_d[:])
            nc.scalar.dma_start(
                out=cwp2[64:128].rearrange("p n t c -> p (n t c)"), in_=cwp_d[:]
            )

            w1v = blob128[:, C_W1 : C_W1 + 512].rearrange("p (c m) -> p c m", c=4)
            w2v = blob128[:, C_W2 : C_W2 + 64]
            rvTv = blob128[:, C_RVT : C_RVT + 16].rearrange("p (c b) -> p c b", c=4)
            b1v = blob128[:, C_B1 : C_B1 + 1]
            b2v = blob128[0:EDIM, C_B2 : C_B2 + 1]
            embTv = blob128[0:EDIM, C_EMBT : C_EMBT + NB]
            id4 = blob128[0:4, C_ID4 : C_ID4 + 4]
            cbA = blob10[:, 0:64]
            cbB = blob10[:, 64:128]
            selA = blob10[0:4, 128:256]
            selB = blob10[0:4, 256:384]

            # ---------- emb normalization (independent of rv) ----------
            esq = work.tile([EDIM, NB], F32, tag="esq")
            nc.vector.tensor_mul(esq[:], embTv, embTv)
            nsqE = pspool.tile([1, NB], F32, tag="small")
            nc.tensor.matmul(nsqE[:], ones64[:], esq[:], start=True, stop=True)
            enrm = work.tile([1, NB], F32, tag="enrm")
            nc.scalar.activation(out=enrm[:], in_=nsqE[:], func=AF.Sqrt)
            einv = work.tile([1, NB], F32, tag="einv")
            nc.vector.reciprocal(einv[:], enrm[:])
            ebc = pspool.tile([EDIM, NB], F32, tag="small")
            nc.tensor.matmul(ebc[:], onesR[:], einv[:], start=True, stop=True)
            embnT = work.tile([EDIM, NB], F32, tag="embnT")
            nc.vector.tensor_mul(embnT[:], embTv, ebc[:])

            # ---------- routing MLP ----------
            h1 = pspool.tile([HID, BLOC], F32, tag="small")
            for c in range(4):
                nc.tensor.matmul(
                    h1[:], w1v[:, c, :], rvTv[:, c, :], start=(c == 0), stop=(c == 3)
                )
            h1r = work.tile([HID, BLOC], F32, tag="h1r")
            nc.scalar.activation(
                out=h1r[:], in_=h1[:], func=AF.Relu, bias=b1v, scale=1.0
            )
            rps = pspool.tile([EDIM, BLOC], F32, tag="small")
            nc.tensor.matmul(rps[:], w2v, h1r[:], start=True, stop=True)
            rsb = work.tile([EDIM, BLOC], F32, tag="rsb")
            nc.scalar.activation(
                out=rsb[:], in_=rps[:], func=AF.Identity, bias=b2v, scale=1.0
            )

            # ---------- r norm + cosine sim + softmax ----------
            rsq = work.tile([EDIM, BLOC], F32, tag="rsq")
            nc.vector.tensor_mul(rsq[:], rsb[:], rsb[:])
            nsq = pspool.tile([BLOC, 1], F32, tag="small")
            nc.tensor.matmul(nsq[:], rsq[:], ones64[:], start=True, stop=True)
            rnrm = work.tile([BLOC, 1], F32, tag="rnrm")
            nc.scalar.activation(out=rnrm[:], in_=nsq[:], func=AF.Sqrt)
            rinv = work.tile([BLOC, 1], F32, tag="rinv")
            nc.vector.reciprocal(rinv[:], rnrm[:])

            simps = pspool.tile([BLOC, NB], F32, tag="small")
            nc.tensor.matmul(simps[:], rsb[:], embnT[:], start=True, stop=True)
            # |cosine| <= 1 so exp() is safe without max subtraction
            ex = work.tile([BLOC, NB], F32, tag="ex")
            nc.scalar.activation(out=ex[:], in_=simps[:], func=AF.Exp, scale=rinv[:])
            s = work.tile([BLOC, 1], F32, tag="s")
            nc.vector.tensor_reduce(s[:], ex[:], axis=AX.X, op=ALU.add)
            sinv = work.tile([BLOC, 1], F32, tag="sinv")
            nc.vector.reciprocal(sinv[:], s[:])
            wf = work.tile([BLOC, NB], F32, tag="wf")
            nc.vector.tensor_scalar_mul(out=wf[:], in0=ex[:], scalar1=sinv[:])

            # ---------- wfT / per-partition weight broadcast ----------
            wfT_ps = pspool.tile([NB, BLOC], F32, tag="small")
            nc.tensor.transpose(wfT_ps[:], wf[:], id4)
            wfT = work.tile([NB, BLOC], F32, tag="wfT")
            nc.scalar.copy(out=wfT[:], in_=wfT_ps[:])

            wfbc = []
            for j, sel in enumerate((selA, selB)):
                ps = pspool.tile([128, NB], F32, tag="small")
                nc.tensor.matmul(ps[:], sel, wf[:], start=True, stop=True)
                t = consts.tile([128, NB], F32, tag=f"wfbc{j}")
                nc.vector.tensor_scalar_mul(out=t[:], in0=ps[:], scalar1=1.0)
                wfbc.append(t)

            # ---------- PE warmup (fills weff window, warms HAM) ----------
            cwpf = cwp2[:].rearrange("p n t c -> p (n t c)")
            warm_ps = pswarm.tile([128, 256], F32, tag="warm")
            for _ in range(NWARM):
                nc.tensor.matmul(
                    warm_ps[:], cwpf[:, 0:128], cwpf[:, 256:512], start=True, stop=True
                )

            # ---------- drain biases biasM[128, 6] ----------
            # col: 0=[s0|s2] 1=[s1|s3] 2=[s2|s2] 3=[s3|s3] 4=[s0|s0] 5=[s1|s1]
            bps = pspool.tile([128, 6], F32, tag="small")
            nc.tensor.matmul(
                bps[0:64, 0:4], cbA, wfT[:, 0:4], start=True, stop=True,
                tile_position=(0, 0),
            )
            nc.tensor.matmul(
                bps[0:64, 4:6], cbA, wfT[:, 0:2], start=True, stop=True,
                tile_position=(0, 0),
            )
            nc.tensor.matmul(
                bps[64:128, 0:2], cbB, wfT[:, 2:4], start=True, stop=True,
                tile_position=(0, 64),
            )
            nc.tensor.matmul(
                bps[64:128, 2:4], cbB, wfT[:, 2:4], start=True, stop=True,
                tile_position=(0, 64),
            )
            nc.tensor.matmul(
                bps[64:128, 4:6], cbB, wfT[:, 0:2], start=True, stop=True,
                tile_position=(0, 64),
            )
            biasM = consts.tile([128, 6], F32, tag="biasM")
            nc.scalar.copy(out=biasM[:], in_=bps[:])

            warm_sink = work.tile([1, 1], F32, tag="warm_sink")
            nc.scalar.copy(out=warm_sink[:], in_=warm_ps[0:1, 0:1])

            # ---------- effective conv weights (DVE, fp16) ----------
            weff = []
            for j in range(2):
                t = consts.tile([128, NTAP, COUT], F16, tag=f"weff{j}")
                nc.vector.tensor_scalar_mul(
                    out=t[:], in0=cwp2[:, 0], scalar1=wfbc[j][:, 0:1]
                )
                for n in range(1, NB):
                    nc.vector.scalar_tensor_tensor(
                        out=t[:],
                        in0=cwp2[:, n],
                        scalar=wfbc[j][:, n : n + 1],
                        in1=t[:],
                        op0=ALU.mult,
                        op1=ALU.add,
                    )
                weff.append(t)

            # ---------- conv: 7 groups x 9 taps x 4 quadrants ----------
            # group = (X tile idx, chunk A, Y tile idx, chunk B, biasA, biasB)
            groups = [
                (0, 0, 0, 1, 4, 5),
                (0, 2, 0, 3, 4, 5),
                (0, 4, 1, 0, 0, 1),
                (0, 5, 1, 1, 0, 1),
                (0, 6, 1, 2, 0, 1),
                (1, 3, 1, 4, 2, 3),
                (1, 5, 1, 6, 2, 3),
            ]
            for gi, (jx, chA, jy, chB, bcA, bcB) in enumerate(groups):
                wX, wY = weff[jx], weff[jy]
                xX, xY = xt[jx], xt[jy]
                psA = psconv.tile([128, NFREE], F32, tag="psA")
                psB = psconv.tile([128, NFREE], F32, tag="psB")
                for t in range(NTAP):
                    offA = chA * CHUNK * W + TAP_OFF[t]
                    offB = chB * CHUNK * W + TAP_OFF[t]
                    st, sp = (t == 0), (t == NTAP - 1)
                    nc.tensor.matmul(
                        psA[0:64], wX[0:64, t], xX[0:64, offA : offA + NFREE],
                        start=st, stop=sp, tile_position=(0, 0),
                    )
                    nc.tensor.matmul(
                        psB[0:64], wX[64:128, t], xX[64:128, offA : offA + NFREE],
                        start=st, stop=sp, tile_position=(64, 0),
                    )
                    nc.tensor.matmul(
                        psA[64:128], wY[0:64, t], xY[0:64, offB : offB + NFREE],
                        start=st, stop=sp, tile_position=(0, 64),
                    )
                    nc.tensor.matmul(
                        psB[64:128], wY[64:128, t], xY[64:128, offB : offB + NFREE],
                        start=st, stop=sp, tile_position=(64, 64),
                    )
                # drain: psA on scalar(ACT), psB on vector(DVE)
                stage = stpool.tile([128, 2, CHUNK, OW], F32, tag="st")
                psAv = psA[:].rearrange("p (r w) -> p r w", w=W)[:, :, 0:OW]
                psBv = psB[:].rearrange("p (r w) -> p r w", w=W)[:, :, 0:OW]
                nc.scalar.activation(
                    out=stage[:, 0], in_=psAv, func=AF.Identity,
                    bias=biasM[:, bcA : bcA + 1], scale=1.0,
                )
                nc.vector.tensor_scalar_add(
                    out=stage[:, 1], in0=psBv, scalar1=biasM[:, bcB : bcB + 1]
                )
                # out DMA: one 4D descriptor per partition-half
                sX0 = 2 * jx  # sample of X half0 (s0 or s2)
                sY0 = 2 * jy
                oA = out_d[sX0 : sX0 + 2, :, chA * CHUNK : chA * CHUNK + CHUNK, :]
                oB = out_d[sY0 : sY0 + 2, :, chB * CHUNK : chB * CHUNK + CHUNK, :]
                nc.sync.dma_start(
                    out=oA.rearrange("s c r w -> c s r w"), in_=stage[0:64]
                )
                nc.sync.dma_start(
                    out=oB.rearrange("s c r w -> c s r w"), in_=stage[64:128]
                )

    fix_sync_waits(nc)
    return nc


_NC = None


def _get_nc():
    global _NC
    if _NC is None:
        _NC = build()
    return _NC


def make_in_maps(inputs):
    x = np.asarray(inputs["x"], dtype=np.float32)
    rvec = np.asarray(inputs["routing_vector"], dtype=np.float32)
    W1 = np.asarray(inputs["W1"], dtype=np.float32)
    b1 = np.asarray(inputs["b1"], dtype=np.float32)
    W2 = np.asarray(inputs["W2"], dtype=np.float32)
    b2 = np.asarray(inputs["b2"], dtype=np.float32)
    emb = np.asarray(inputs["emb"], dtype=np.float32)
    conv_w = np.asarray(inputs["conv_w"], dtype=np.float32)
    conv_b = np.asarray(inputs["conv_b"], dtype=np.float32)

    x16 = np.ascontiguousarray(
        x.reshape(NCORES, BLOC, CIN, HW).astype(np.float16)
    )
    # conv_w[n, co, ci, ky, kx] -> [ci, n, tap, co] fp16
    cwp = np.ascontiguousarray(
        conv_w.transpose(2, 0, 3, 4, 1).reshape(CIN, NB * NTAP * COUT)
    ).astype(np.float16)

    blob = np.zeros((128, NCOL128), np.float32)
    blob[:, C_W1 : C_W1 + 512] = (
        W1.reshape(4, 128, HID).transpose(1, 0, 2).reshape(128, 512)
    )
    blob[:, C_W2 : C_W2 + 64] = W2
    blob[:, C_B1] = b1
    blob[0:EDIM, C_B2] = b2
    blob[0:EDIM, C_EMBT : C_EMBT + NB] = emb.T
    blob[0:4, C_ID4 : C_ID4 + 4] = np.eye(4, dtype=np.float32)

    blob10 = np.zeros((NB, 384), np.float32)
    blob10[:, 0:64] = conv_b
    blob10[:, 64:128] = conv_b
    sel = np.zeros((2, 4, 128), np.float32)
    for j in range(2):
        sel[j, 2 * j, 0:64] = 1.0
        sel[j, 2 * j + 1, 64:128] = 1.0
    blob10[0:4, 128:256] = sel[0]
    blob10[0:4, 256:384] = sel[1]

    in_maps = []
    for c in range(NCORES):
        bc = blob.copy()
        rvc = rvec[BLOC * c : BLOC * (c + 1)]  # [4, 512]
        bc[:, C_RVT : C_RVT + 16] = (
            rvc.T.reshape(4, 128, BLOC).transpose(1, 0, 2).reshape(128, 16)
        )
        in_maps.append(
            {
                "x": x16[c],
                "cwp": cwp,
                "blob128": np.ascontiguousarray(bc),
                "blob10": blob10,
            }
        )
    return in_maps


def kernel(**inputs):
    from concourse.bass_utils import run_bass_kernel_spmd

    nc = _get_nc()
    in_maps = make_in_maps(inputs)
    res = run_bass_kernel_spmd(nc, in_maps, core_ids=list(range(NCORES)))
    return np.concatenate([r["out"] for r in res.results], axis=0)
